# revision 1
# baseline (speedup 1.0000x reference)
"""Trainium2 Bass kernel for nn_ExtractorMLP (gather + 3-layer edge MLP).

Strategy
--------
Edges are sharded contiguously across 8 cores (100k each). Per core, edges are
partitioned into 4 static segments by (col>=32768, row>=32768) so all gather
indices fit int16 (dma_gather requirement); each segment gathers from a
statically-offset slice of the node table (full f32 emb, 256B rows).

Per 2048-edge gather group: two non-transpose dma_gathers (col, row) land
[128 edges x 64 feats] f32 subtiles (edge-major). SWDGE descriptor generation
is the measured bottleneck (~33ns/desc/queue), so gathers round-robin all 4
SWDGE queues. Each 128-edge subtile is PE-transposed (via identity) into a
[128, 512] PSUM tile: col features on partitions 0-63, row on 64-127 -- the
feature-major layout the PE contracts over. MLP is then exact f32:
  h1T[256,E]: 2 matmuls (lhsT = W1 halves, K=128 = col|row feats)
  s1 = relu(h1T + b1) on ScalarE; h2T[64,E]: 2 matmuls (K=256 split);
  s2 = relu(h2T + b2); out[1,E]: matmul vs W3; +b3 on VectorE; DMA out.
"""

import numpy as np

import concourse.bacc as bacc
import concourse.bass as bass
import concourse.mybir as mybir
import concourse.tile as tile
import concourse.tile_sem_assignment as _tsa
from concourse.bass_utils import run_bass_kernel_spmd

# Tile assigns DMASW sem lanes round-robin in scheduled order, while the sim /
# ucode lock each lane to a single SWDGE queue.  With multi-queue gathers the
# blind rotation mixes queues on one lane.  Pin lanes by queue: queue q owns
# lanes {2q, 2q+1} (8 lanes / 4 queues), toggling for pipelining.
_orig_assign_tick = _tsa.TileClockTick._assign_tick


def _queue_affine_assign_tick(self, inst):
    if (
        isinstance(inst, _tsa.DMAInst)
        and getattr(inst, "engine", None) == mybir.EngineType.Pool
        and getattr(inst, "queue_num", None) is not None
    ):
        q = inst.queue_num
        tog = getattr(self, "_q_lane_toggle", None)
        if tog is None:
            tog = self._q_lane_toggle = {}
        t = tog.get(q, 0)
        tog[q] = t ^ 1
        self.next_sw_dma_idx = 2 * q + t
    return _orig_assign_tick(self, inst)


_tsa.TileClockTick._assign_tick = _queue_affine_assign_tick

N_NODES = 50000
N_EDGES = 800000
HID = 64
NCORES = 8
EPC = N_EDGES // NCORES          # edges per core
TILE_E = 512                     # edges per compute tile
SPLIT = 32768                    # int16 index split point
SEG_CAP_TILES = [88, 48, 48, 28]  # caps (tile counts); actual max [85,45,45,24]
T_TOTAL = sum(SEG_CAP_TILES)     # tiles per core
GROUP = 4                        # tiles per dma_gather (2048 idxs; >2048 is unstable)

_SEG_BASE = [(0, 0), (0, SPLIT), (SPLIT, 0), (SPLIT, SPLIT)]


MAC_E = 1024                      # edges per macro-tile (= 2 base tiles)
N_MACROS = T_TOTAL // 2


def build_nc(repeat: int = 1):
    """Build + compile the per-core bass program. Same program for all cores.

    Platform note: this axon runtime costs ~7-9us per *instruction* and ~70us
    per *blocking* cross-engine wait, so the kernel uses fat 1024-edge
    macro-tiles (fewest instructions) and deep buffering (pre-satisfied waits).
    """
    f32 = mybir.dt.float32
    i16 = mybir.dt.int16

    nc = bacc.Bacc("TRN2", target_bir_lowering=False, debug=False,
                   num_swdge_queues=4)

    embf = nc.dram_tensor("embf", [N_NODES, HID], f32, kind="ExternalInput")
    colidx = nc.dram_tensor("colidx", [128, T_TOTAL * 32], i16, kind="ExternalInput")
    rowidx = nc.dram_tensor("rowidx", [128, T_TOTAL * 32], i16, kind="ExternalInput")
    w1 = nc.dram_tensor("w1", [128, 256], f32, kind="ExternalInput")
    w2 = nc.dram_tensor("w2", [128, 2 * HID], f32, kind="ExternalInput")
    w3 = nc.dram_tensor("w3", [HID, 1], f32, kind="ExternalInput")
    b1d = nc.dram_tensor("b1", [128, 2], f32, kind="ExternalInput")
    b2d = nc.dram_tensor("b2", [HID, 1], f32, kind="ExternalInput")
    b3d = nc.dram_tensor("b3", [1, 1], f32, kind="ExternalInput")
    identd = nc.dram_tensor("ident", [128, 128], f32, kind="ExternalInput")
    out = nc.dram_tensor("out", [N_MACROS, MAC_E], f32, kind="ExternalOutput")

    # macro groups: (macro_idx, seg); segment caps are even so macros align
    macros = []
    t0 = 0
    for s, n in enumerate(SEG_CAP_TILES):
        assert n % 2 == 0 or s == 3, (s, n)
        for m in range(n // 2):
            macros.append((t0 // 2 + m, s))
        t0 += n

    relu = mybir.ActivationFunctionType.Relu
    SUB = MAC_E // 128  # 8 subtiles of 128 edges per macro

    with tile.TileContext(nc) as tc:
        with (
            tc.tile_pool(name="const", bufs=1) as cpool,
            tc.tile_pool(name="gath", bufs=3) as gpool,
            tc.tile_pool(name="act", bufs=3) as apool,
            tc.tile_pool(name="ps_t", bufs=2, space="PSUM") as ppool_t,
            tc.tile_pool(name="ps_w", bufs=2, space="PSUM") as ppool_w,
        ):
            cix = cpool.tile([128, T_TOTAL * 32], i16)
            rix = cpool.tile([128, T_TOTAL * 32], i16)
            w1s = cpool.tile([128, 256], f32)
            w2s = cpool.tile([128, 2 * HID], f32)
            w3s = cpool.tile([HID, 1], f32)
            b1s = cpool.tile([128, 2], f32)
            b2s = cpool.tile([HID, 1], f32)
            b3s = cpool.tile([1, 1], f32)
            idn = cpool.tile([128, 128], f32)
            nc.sync.dma_start(cix[:], colidx[:])
            nc.sync.dma_start(rix[:], rowidx[:])
            nc.sync.dma_start(w1s[:], w1[:])
            nc.sync.dma_start(w2s[:], w2[:])
            nc.sync.dma_start(w3s[:], w3[:])
            nc.sync.dma_start(b1s[:], b1d[:])
            nc.sync.dma_start(b2s[:], b2d[:])
            nc.sync.dma_start(b3s[:], b3d[:])
            nc.sync.dma_start(idn[:], identd[:])

            # Software-pipelined emission: stages skewed across macros so every
            # engine's static stream interleaves macros and all cross-engine
            # waits are pre-satisfied by the time they are reached.
            state = {}   # macro idx -> dict of tiles
            qq = [0]

            def st_gather(m):
                mi, s = macros[m]
                cbase, rbase = _SEG_BASE[s]
                ix0 = mi * 2 * 32
                comb = gpool.tile([128, 2 * SUB, HID], f32, tag="comb")
                q = qq[0]
                nc.gpsimd.dma_gather(
                    comb[:, 0:SUB, :], embf[cbase:, :],
                    cix[:, ix0:ix0 + 64], MAC_E, MAC_E, HID,
                    transpose=False, queue_num=q % 4, single_packet=False)
                nc.gpsimd.dma_gather(
                    comb[:, SUB:2 * SUB, :], embf[rbase:, :],
                    rix[:, ix0:ix0 + 64], MAC_E, MAC_E, HID,
                    transpose=False, queue_num=(q + 1) % 4, single_packet=False)
                qq[0] = q + 2
                state[m] = {"comb": comb}

            def st_interleave(m):
                d = state[m]
                ci = apool.tile([128, 2 * SUB, HID], f32, tag="ci")
                nc.vector.tensor_copy(ci[:, 0::2, :], d["comb"][:, 0:SUB, :])
                nc.vector.tensor_copy(ci[:, 1::2, :], d["comb"][:, SUB:2 * SUB, :])
                d["ci"] = ci

            def st_transpose(m):
                d = state[m]
                tp = ppool_t.tile([128, MAC_E], f32, tag="tp")
                for k in range(SUB):
                    nc.tensor.transpose(
                        tp[:, k * 128:(k + 1) * 128],
                        d["ci"][:, 2 * k:2 * k + 2, :], idn[:])
                g32 = apool.tile([128, MAC_E], f32, tag="g32")
                nc.scalar.copy(g32[:], tp[:])
                d["g32"] = g32

            def st_l1(m):
                d = state[m]
                g32 = d["g32"]
                h1a = ppool_w.tile([128, 2, 512], f32, tag="work")
                nc.tensor.matmul(h1a[:, 0, :], w1s[:, 0:128], g32[:, 0:512], start=True, stop=True)
                nc.tensor.matmul(h1a[:, 1, :], w1s[:, 0:128], g32[:, 512:1024], start=True, stop=True)
                h1b = ppool_w.tile([128, 2, 512], f32, tag="work")
                nc.tensor.matmul(h1b[:, 0, :], w1s[:, 128:256], g32[:, 0:512], start=True, stop=True)
                nc.tensor.matmul(h1b[:, 1, :], w1s[:, 128:256], g32[:, 512:1024], start=True, stop=True)
                s1a = apool.tile([128, MAC_E], f32, tag="s1a")
                nc.scalar.activation(s1a[:], h1a[:].rearrange("p a b -> p (a b)"), relu, bias=b1s[:, 0:1])
                s1b = apool.tile([128, MAC_E], f32, tag="s1b")
                nc.scalar.activation(s1b[:], h1b[:].rearrange("p a b -> p (a b)"), relu, bias=b1s[:, 1:2])
                d["s1a"], d["s1b"] = s1a, s1b

            def st_l2(m):
                d = state[m]
                h2 = ppool_w.tile([128, 2, 512], f32, tag="work")
                for j in range(2):
                    nc.tensor.matmul(h2[0:HID, j, :], w2s[:, 0:HID],
                                     d["s1a"][:, j * 512:(j + 1) * 512], start=True, stop=False)
                    nc.tensor.matmul(h2[0:HID, j, :], w2s[:, HID:2 * HID],
                                     d["s1b"][:, j * 512:(j + 1) * 512], start=False, stop=True)
                s2 = apool.tile([HID, MAC_E], f32, tag="s2")
                nc.scalar.activation(s2[:], h2[0:HID, :, :].rearrange("p a b -> p (a b)"), relu, bias=b2s[:])
                d["s2"] = s2

            def st_l3(m):
                d = state[m]
                mi, _ = macros[m]
                o = ppool_w.tile([128, 2, 512], f32, tag="work")
                for j in range(2):
                    nc.tensor.matmul(o[0:1, j, :], w3s[:],
                                     d["s2"][:, j * 512:(j + 1) * 512], start=True, stop=True)
                stage = apool.tile([1, MAC_E], f32, tag="stage")
                nc.vector.tensor_scalar_add(
                    stage[:], o[0:1, :, :].rearrange("p a b -> p (a b)"), b3s[0:1, 0:1])
                nc.sync.dma_start(out[mi:mi + 1, :], stage[:])
                del state[m]

            stages = [st_gather, st_interleave, st_transpose, st_l1, st_l2, st_l3]
            nm = len(macros)
            for _rep in range(repeat):
                for i in range(nm + len(stages) - 1):
                    for si in range(len(stages) - 1, -1, -1):
                        m = i - si
                        if 0 <= m < nm:
                            stages[si](m)

    nc.compile()
    return nc


def _wrap16(arr_t512):
    """[T*512] int16 -> [128, T*32] wrapped-by-16 idx layout, replicated x8."""
    T = arr_t512.shape[0] // TILE_E
    a = arr_t512.reshape(T, 32, 16).transpose(2, 0, 1).reshape(16, T * 32)
    return np.tile(a, (8, 1)).astype(np.int16)


def prep_inputs(emb, edge_index, W1, b1, W2, b2, W3, b3):
    """Host-side marshalling. Returns (in_maps, origpos_per_core)."""
    emb = np.ascontiguousarray(np.asarray(emb, np.float32))
    ei = np.asarray(edge_index).astype(np.int64)
    W1 = np.asarray(W1, np.float32)
    b1 = np.asarray(b1, np.float32)
    W2 = np.asarray(W2, np.float32)
    b2 = np.asarray(b2, np.float32)
    W3 = np.asarray(W3, np.float32)
    b3 = np.asarray(b3, np.float32)

    w2p = np.ascontiguousarray(np.concatenate([W2[0:128, :], W2[128:256, :]], axis=1)).astype(np.float32)
    b1p = np.ascontiguousarray(np.stack([b1[0:128], b1[128:256]], axis=1)).astype(np.float32)
    ident = np.eye(128, dtype=np.float32)

    in_maps = []
    origpos = []
    for c in range(NCORES):
        sl = slice(c * EPC, (c + 1) * EPC)
        col = ei[0, sl]
        row = ei[1, sl]
        seg = (col >= SPLIT) * 2 + (row >= SPLIT)
        cloc = np.zeros(T_TOTAL * TILE_E, np.int16)
        rloc = np.zeros(T_TOTAL * TILE_E, np.int16)
        orig = np.full(T_TOTAL * TILE_E, -1, np.int64)
        off = 0
        for s in range(4):
            m = np.nonzero(seg == s)[0]
            n = len(m)
            cap = SEG_CAP_TILES[s] * TILE_E
            assert n <= cap, f"core {c} segment {s}: {n} > cap {cap}"
            cloc[off:off + n] = (col[m] - _SEG_BASE[s][0]).astype(np.int16)
            rloc[off:off + n] = (row[m] - _SEG_BASE[s][1]).astype(np.int16)
            orig[off:off + n] = c * EPC + m
            off += cap
        in_maps.append({
            "embf": emb,
            "colidx": _wrap16(cloc),
            "rowidx": _wrap16(rloc),
            "w1": np.ascontiguousarray(W1),
            "w2": w2p,
            "w3": np.ascontiguousarray(W3),
            "b1": b1p,
            "b2": np.ascontiguousarray(b2[:, None]),
            "b3": b3.reshape(1, 1),
            "ident": ident,
        })
        origpos.append(orig)
    return in_maps, origpos


def unshard(results, origpos):
    out_full = np.empty((N_EDGES, 1), np.float32)
    for c in range(NCORES):
        vals = results[c]["out"].reshape(-1)
        orig = origpos[c]
        valid = orig >= 0
        out_full[orig[valid], 0] = vals[valid]
    return out_full


_NC_CACHE = {}


def _get_nc(repeat: int = 1):
    if repeat not in _NC_CACHE:
        _NC_CACHE[repeat] = build_nc(repeat)
    return _NC_CACHE[repeat]


def kernel(**inputs) -> np.ndarray:
    nc = _get_nc(1)
    in_maps, origpos = prep_inputs(
        inputs["emb"], inputs["edge_index"],
        inputs["W1"], inputs["b1"], inputs["W2"], inputs["b2"],
        inputs["W3"], inputs["b3"])
    res = run_bass_kernel_spmd(nc, in_maps, core_ids=list(range(NCORES)))
    return unshard(res.results, origpos)



# revision 2
# speedup vs baseline: 2.9948x; 2.9948x over previous
"""Trainium2 Bass kernel v2 for nn_ExtractorMLP: hybrid waves.

Edges sharded across 8 cores (100k each), padded to 98 macro-tiles of 1024.
No quadrant segmentation: gather base = embf[32768:] and idx16 = node - 32768
(SWDGE sign-extends mid-list negatives; host swaps an edge with both
endpoints >= 32768 into each macro's last slot since trailing negatives are
treated as padding and skipped).

Structure per repeat: 7 waves x [28 unrolled dma_gathers (14 macros x 2, 1024
idx each, 4 SWDGE queues) -> For_i(0,14) compute loop]. The compute loop body
(26 insts) reads the wave buffer with loop-var slicing: interleave (2 DVE),
8 PE transposes + PSUM->SBUF copy, L1 4 matmuls + 2 relu acts, L2 4 matmuls
+ relu act, L3 2 matmuls + b3 add (PSUM drain), out DMA at dynamic offset.
Gathers stay OUT of dynamic loops (in-loop gather costs ~18ms/trip flat).
"""

import numpy as np

import concourse.bacc as bacc
import concourse.bass as bass
import concourse.mybir as mybir
import concourse.tile as tile
import concourse.tile_sem_assignment as _tsa
from concourse.bass_utils import run_bass_kernel_spmd

_orig_assign_tick = _tsa.TileClockTick._assign_tick


def _queue_affine_assign_tick(self, inst):
    if (
        isinstance(inst, _tsa.DMAInst)
        and getattr(inst, "engine", None) == mybir.EngineType.Pool
        and getattr(inst, "queue_num", None) is not None
    ):
        q = inst.queue_num
        tog = getattr(self, "_q_lane_toggle", None)
        if tog is None:
            tog = self._q_lane_toggle = {}
        t = tog.get(q, 0)
        tog[q] = t ^ 1
        self.next_sw_dma_idx = 2 * q + t
    return _orig_assign_tick(self, inst)


_tsa.TileClockTick._assign_tick = _queue_affine_assign_tick

N_NODES = 50000
N_EDGES = 800000
HID = 64
NCORES = 8
EPC = N_EDGES // NCORES           # 100000 edges per core
MAC = 1024                        # edges per macro-tile
NMAC = 98                         # ceil(EPC / MAC) -> 352 pad slots
WAVE = 14                         # macros per wave
NWAVE = NMAC // WAVE              # 7
OFF = 32768                       # idx16 = node - OFF

f32 = mybir.dt.float32
i16 = mybir.dt.int16
relu = mybir.ActivationFunctionType.Relu


def build_nc(repeat: int = 1):
    nc = bacc.Bacc("TRN2", target_bir_lowering=False, debug=False,
                   num_swdge_queues=4)

    embf = nc.dram_tensor("embf", [N_NODES, HID], f32, kind="ExternalInput")
    idxd = nc.dram_tensor("idxd", [128, NMAC, 128], i16, kind="ExternalInput")
    w1d = nc.dram_tensor("w1", [128, 256], f32, kind="ExternalInput")
    w2d = nc.dram_tensor("w2", [128, 2 * HID], f32, kind="ExternalInput")
    w3d = nc.dram_tensor("w3", [HID, 1], f32, kind="ExternalInput")
    b1d = nc.dram_tensor("b1", [128, 2], f32, kind="ExternalInput")
    b2d = nc.dram_tensor("b2", [HID, 1], f32, kind="ExternalInput")
    b3d = nc.dram_tensor("b3", [1, 1], f32, kind="ExternalInput")
    identd = nc.dram_tensor("ident", [128, 128], f32, kind="ExternalInput")
    out = nc.dram_tensor("out", [NMAC, 1, MAC], f32, kind="ExternalOutput")

    with tile.TileContext(nc) as tc:
        with (
            tc.tile_pool(name="const", bufs=1) as cpool,
            tc.tile_pool(name="wave", bufs=2) as wvpool,
            tc.tile_pool(name="act", bufs=2) as apool,
            tc.tile_pool(name="ps_t", bufs=2, space="PSUM") as ppool_t,
            tc.tile_pool(name="ps_w", bufs=2, space="PSUM") as ppool_w,
        ):
            ixall = cpool.tile([128, NMAC, 128], i16)
            w1s = cpool.tile([128, 256], f32)
            w2s = cpool.tile([128, 2 * HID], f32)
            w3s = cpool.tile([HID, 1], f32)
            b1s = cpool.tile([128, 2], f32)
            b2s = cpool.tile([HID, 1], f32)
            b3s = cpool.tile([1, 1], f32)
            idn = cpool.tile([128, 128], f32)
            nc.sync.dma_start(ixall[:], idxd[:])
            nc.sync.dma_start(w1s[:], w1d[:])
            nc.sync.dma_start(w2s[:], w2d[:])
            nc.sync.dma_start(w3s[:], w3d[:])
            nc.sync.dma_start(b1s[:], b1d[:])
            nc.sync.dma_start(b2s[:], b2d[:])
            nc.sync.dma_start(b3s[:], b3d[:])
            nc.sync.dma_start(idn[:], identd[:])

            for _rep in range(repeat):
                for w in range(NWAVE):
                    wb = wvpool.tile([128, WAVE, 16, HID], f32, name="wb",
                                     tag="wb")
                    for k in range(WAVE):
                        mg = w * WAVE + k
                        nc.gpsimd.dma_gather(
                            wb[:, k, 0:8, :], embf[OFF:, :],
                            ixall[:, mg, 0:64], MAC, MAC, HID,
                            transpose=False, queue_num=(2 * k) % 4,
                            single_packet=False)
                        nc.gpsimd.dma_gather(
                            wb[:, k, 8:16, :], embf[OFF:, :],
                            ixall[:, mg, 64:128], MAC, MAC, HID,
                            transpose=False, queue_num=(2 * k + 1) % 4,
                            single_packet=False)

                    with tc.For_i(0, WAVE, 1) as m:
                        # engine APs must be static in-loop (NEFF disables
                        # vector_dynamic_offsets); dyn-offset reads go via DMA
                        ci = apool.tile([128, 16, HID], f32, name="ci")
                        nc.sync.dma_start(ci[:, 0::2, :], wb[:, m, 0:8, :])
                        nc.sync.dma_start(ci[:, 1::2, :], wb[:, m, 8:16, :])
                        tp = ppool_t.tile([128, MAC], f32, name="tp")
                        for k in range(8):
                            nc.tensor.transpose(
                                tp[:, k * 128:(k + 1) * 128],
                                ci[:, 2 * k:2 * k + 2, :], idn[:])
                        g32 = apool.tile([128, MAC], f32, name="g32")
                        nc.scalar.copy(g32[:], tp[:])

                        h1a = ppool_w.tile([128, MAC], f32, name="h1a",
                                           tag="work")
                        h1b = ppool_w.tile([128, MAC], f32, name="h1b",
                                           tag="work")
                        for j in range(2):
                            sl = slice(j * 512, (j + 1) * 512)
                            nc.tensor.matmul(h1a[:, sl], w1s[:, 0:128],
                                             g32[:, sl], start=True, stop=True)
                            nc.tensor.matmul(h1b[:, sl], w1s[:, 128:256],
                                             g32[:, sl], start=True, stop=True)
                        s1a = apool.tile([128, MAC], f32, name="s1a")
                        s1b = apool.tile([128, MAC], f32, name="s1b")
                        nc.scalar.activation(s1a[:], h1a[:], relu,
                                             bias=b1s[:, 0:1])
                        nc.scalar.activation(s1b[:], h1b[:], relu,
                                             bias=b1s[:, 1:2])

                        h2 = ppool_w.tile([HID, MAC], f32, name="h2",
                                          tag="work")
                        for j in range(2):
                            sl = slice(j * 512, (j + 1) * 512)
                            nc.tensor.matmul(h2[:, sl], w2s[:, 0:HID],
                                             s1a[:, sl], start=True,
                                             stop=False)
                            nc.tensor.matmul(h2[:, sl], w2s[:, HID:2 * HID],
                                             s1b[:, sl], start=False,
                                             stop=True)
                        s2 = apool.tile([HID, MAC], f32, name="s2")
                        nc.scalar.activation(s2[:], h2[:], relu, bias=b2s[:])

                        o = ppool_w.tile([1, MAC], f32, name="o", tag="work")
                        for j in range(2):
                            sl = slice(j * 512, (j + 1) * 512)
                            nc.tensor.matmul(o[:, sl], w3s[:], s2[:, sl],
                                             start=True, stop=True)
                        ost = apool.tile([1, MAC], f32, name="ost")
                        nc.vector.tensor_scalar_add(ost[:], o[:],
                                                    b3s[0:1, 0:1])
                        nc.sync.dma_start(out[w * WAVE + m, :, :], ost[:])

    nc.compile()
    return nc


def prep_inputs(emb, edge_index, W1, b1, W2, b2, W3, b3):
    """Host-side marshalling. Returns (in_maps, origpos_per_core)."""
    emb = np.ascontiguousarray(np.asarray(emb, np.float32))
    ei = np.asarray(edge_index).astype(np.int64)
    W1 = np.asarray(W1, np.float32)
    b1 = np.asarray(b1, np.float32)
    W2 = np.asarray(W2, np.float32)
    b2 = np.asarray(b2, np.float32)
    W3 = np.asarray(W3, np.float32)
    b3 = np.asarray(b3, np.float32)

    w2p = np.ascontiguousarray(
        np.concatenate([W2[0:128, :], W2[128:256, :]], axis=1)).astype(np.float32)
    b1p = np.ascontiguousarray(
        np.stack([b1[0:128], b1[128:256]], axis=1)).astype(np.float32)
    ident = np.eye(128, dtype=np.float32)

    in_maps = []
    origpos = []
    for c in range(NCORES):
        sl = slice(c * EPC, (c + 1) * EPC)
        col = ei[0, sl].copy()
        row = ei[1, sl].copy()
        orig = np.arange(c * EPC, (c + 1) * EPC, dtype=np.int64)
        # pad to NMAC*MAC with safe high-node edges (output discarded)
        npad = NMAC * MAC - EPC
        col = np.concatenate([col, np.full(npad, N_NODES - 1, np.int64)])
        row = np.concatenate([row, np.full(npad, N_NODES - 1, np.int64)])
        orig = np.concatenate([orig, np.full(npad, -1, np.int64)])

        # per macro: last slot must have BOTH endpoints >= OFF (trailing
        # negatives after the -OFF shift are treated as padding by SWDGE)
        col2 = col.reshape(NMAC, MAC)
        row2 = row.reshape(NMAC, MAC)
        orig2 = orig.reshape(NMAC, MAC)
        for mval in range(NMAC):
            if col2[mval, -1] >= OFF and row2[mval, -1] >= OFF:
                continue
            cand = np.nonzero((col2[mval] >= OFF) & (row2[mval] >= OFF))[0]
            assert len(cand) > 0, f"core {c} macro {mval}: no high-high edge"
            j = cand[0]
            for arr in (col2, row2, orig2):
                arr[mval, -1], arr[mval, j] = arr[mval, j], arr[mval, -1]

        idx = np.empty((128, NMAC, 128), np.int16)
        for mval in range(NMAC):
            cw = _wrap16((col2[mval] - OFF).astype(np.int16))
            rw = _wrap16((row2[mval] - OFF).astype(np.int16))
            idx[:, mval, 0:64] = cw
            idx[:, mval, 64:128] = rw

        in_maps.append({
            "embf": emb,
            "idxd": idx,
            "w1": np.ascontiguousarray(W1),
            "w2": w2p,
            "w3": np.ascontiguousarray(W3),
            "b1": b1p,
            "b2": np.ascontiguousarray(b2[:, None]),
            "b3": b3.reshape(1, 1),
            "ident": ident,
        })
        origpos.append(orig2.reshape(-1))
    return in_maps, origpos


def _wrap16(a1024):
    a = a1024.reshape(64, 16).T.reshape(16, 64)
    return np.tile(a, (8, 1)).astype(np.int16)


def unshard(results, origpos):
    out_full = np.empty((N_EDGES, 1), np.float32)
    for c in range(NCORES):
        vals = results[c]["out"].reshape(-1)
        orig = origpos[c]
        valid = orig >= 0
        out_full[orig[valid], 0] = vals[valid]
    return out_full


_NC_CACHE = {}


def _get_nc(repeat: int = 1):
    if repeat not in _NC_CACHE:
        _NC_CACHE[repeat] = build_nc(repeat)
    return _NC_CACHE[repeat]


def kernel(**inputs) -> np.ndarray:
    nc = _get_nc(1)
    in_maps, origpos = prep_inputs(
        inputs["emb"], inputs["edge_index"],
        inputs["W1"], inputs["b1"], inputs["W2"], inputs["b2"],
        inputs["W3"], inputs["b3"])
    res = run_bass_kernel_spmd(nc, in_maps, core_ids=list(range(NCORES)))
    return unshard(res.results, origpos)
